# revision 25
# baseline (speedup 1.0000x reference)
"""
CosmosUnpatcher3d (inverse 3D Haar wavelet, PATCH_SIZE=2) on 8 Trainium2
NeuronCores.

Math: input  x[b, ch, i, j, k] with ch = 3*g + c, g = (bt, bh, bw) bits
      output y[b, c, t, h, w]  with t = 2i+dt, h = 2j+dh, w = 2k+dw
      y = sum_g (-1)^(bt*dt + bh*dh + bw*dw) * x[...]
(the Haar taps (1/sqrt2)^3 times the final sqrt(8) rescale cancel to
exactly 1.0), then the t=0 plane is dropped (17 output t-planes).

This is an 8-point Hadamard transform across the 8 subband planes,
done as a 3-stage butterfly. On this backend per-instruction overhead
dominates, so the kernel is built from the fewest, largest possible
instructions: each butterfly stage is exactly 2 DVE tensor_tensor ops
(one add, one subtract) over multi-plane strided views.

Sharding: 8 cores = batch(2) x H-quarters(4). Each core processes its
(24, 9, 64, 256) shard in 4 double-buffered rounds:
  round = [128 partitions, 8 planes x 864] resident in SBUF
  in-DMA (3.54 MB) -> 14 flat tensor_tensor ops -> out-DMA (3.54 MB)
All ops use flat contiguous slices (strided multi-dim APs are slow on
this backend). Host packs shards partition-major (pure data movement;
all arithmetic happens on device) and scatters the 8 result planes
into the strided output positions.
"""

import numpy as np

_N_CORES = 8
_B, _CH, _TI, _HI, _WI = 2, 24, 9, 256, 256
_C_OUT = 3
_JQ = 4               # H-quarter cores per batch entry
_HJ = _HI // _JQ      # 64 input rows per core
_PL = 1728            # per-plane elems per partition (3*9*32*256 / 128)
_F = 8 * _PL          # free-dim elems per partition per round

_cached = {}


def _build_nc(repeat=1):
    import concourse.bacc as bacc
    import concourse.mybir as mybir
    from concourse.tile import TileContext
    from concourse.mybir import AluOpType
    from contextlib import ExitStack

    import os

    f32 = mybir.dt.float32
    add, sub = AluOpType.add, AluOpType.subtract
    nc = bacc.Bacc()

    NBUF = int(os.environ.get("K_BUFS", "2"))
    NR = int(os.environ.get("K_ROUNDS", "4"))       # rounds (2, 4, or 8)
    OUT_SYNC = os.environ.get("K_OUT_SYNC", "0") == "1"
    FR = _F * 2 // NR                                # free elems per round
    H, Q, E = FR // 2, FR // 4, FR // 8
    X = nc.declare_dram_parameter("x", [NR, 128, FR], f32, isOutput=False)
    O = nc.declare_dram_parameter("out", [NR, 128, FR], f32, isOutput=True)

    with TileContext(nc) as tc, ExitStack() as ctx:
        pa = ctx.enter_context(tc.tile_pool(name="pa", bufs=NBUF))
        pb = ctx.enter_context(tc.tile_pool(name="pb", bufs=NBUF))
        pc = ctx.enter_context(tc.tile_pool(name="pc", bufs=NBUF))

        for _rep in range(repeat):
            for rr in range(NR):
                t0 = pa.tile([128, FR], f32, tag="a")
                nc.sync.dma_start(out=t0[:], in_=X[rr])
                s1 = pb.tile([128, FR], f32, tag="b")
                # stage 1 (bt -> dt): planes {0..3} vs {4..7} — flat
                nc.vector.tensor_tensor(s1[:, 0:H], t0[:, 0:H], t0[:, H:FR], add)
                nc.vector.tensor_tensor(s1[:, H:FR], t0[:, 0:H], t0[:, H:FR], sub)
                # stage 2 (bh -> dh): within each dt half, {0,1} vs {2,3}
                s2 = pa.tile([128, FR], f32, tag="a")  # reuses t0's slot
                for dt in range(2):
                    b0 = dt * H
                    nc.vector.tensor_tensor(
                        s2[:, b0 : b0 + Q], s1[:, b0 : b0 + Q],
                        s1[:, b0 + Q : b0 + H], add,
                    )
                    nc.vector.tensor_tensor(
                        s2[:, b0 + Q : b0 + H], s1[:, b0 : b0 + Q],
                        s1[:, b0 + Q : b0 + H], sub,
                    )
                # stage 3 (bw -> dw): within each (dt,dh) pair, even vs odd
                if os.environ.get("K_ZB", "0") == "1":
                    z = pb.tile([128, FR], f32, tag="b")
                else:
                    z = pc.tile([128, FR], f32, tag="c")
                for qb in range(4):
                    b0 = qb * Q
                    nc.vector.tensor_tensor(
                        z[:, b0 : b0 + E], s2[:, b0 : b0 + E],
                        s2[:, b0 + E : b0 + Q], add,
                    )
                    nc.vector.tensor_tensor(
                        z[:, b0 + E : b0 + Q], s2[:, b0 : b0 + E],
                        s2[:, b0 + E : b0 + Q], sub,
                    )
                eng = nc.sync if OUT_SYNC else nc.scalar
                eng.dma_start(out=O[rr], in_=z[:])
    nc.finalize()
    return nc


def _get_nc():
    import os

    rep = int(os.environ.get("K_NC_REPEAT", "1"))
    key = ("nc", rep)
    if key not in _cached:
        _cached[key] = _build_nc(rep)
    return _cached[key]


def _nr():
    import os

    return int(os.environ.get("K_ROUNDS", "4"))


def _pack_core(xb, jq):
    """xb: (24, 9, 256, 256) one batch entry; -> (NR, 128, FR) packed."""
    nr = _nr()
    e = 2 * _PL // nr                                    # per-plane per round
    xs = xb[:, :, jq * _HJ : (jq + 1) * _HJ, :]          # (24, 9, 64, 256)
    v = xs.reshape(8, 3, _TI, 2, 32, 256)                # (g, c, i, jc, jl, k)
    v = v.transpose(3, 1, 2, 4, 5, 0)                    # (jc, c, i, jl, k, g)
    v = np.ascontiguousarray(v).reshape(nr, 128, e, 8)   # (round, p, r, g)
    v = v.transpose(0, 1, 3, 2)                          # (round, p, g, r)
    return np.ascontiguousarray(v).reshape(nr, 128, 8 * e)


def kernel(hidden_states: np.ndarray) -> np.ndarray:
    import os
    from concourse.bass_utils import run_bass_kernel_spmd

    x = np.ascontiguousarray(hidden_states, dtype=np.float32)
    assert x.shape == (_B, _CH, _TI, _HI, _WI), x.shape

    nc = _get_nc()
    in_maps = [
        {"x": _pack_core(x[b], jq)} for b in range(_B) for jq in range(_JQ)
    ]
    kw = {}
    if os.environ.get("KTRACE"):
        kw = dict(trace=True, trace_cores=[0])
    res = run_bass_kernel_spmd(nc, in_maps, list(range(_N_CORES)), **kw)
    _cached["last"] = res

    out = np.empty((_B, _C_OUT, 2 * _TI - 1, 2 * _HI, 2 * _WI), dtype=np.float32)
    tmp = np.empty((_C_OUT, 2 * _TI, 2 * _HJ, 2 * _WI), dtype=np.float32)
    nr = _nr()
    e = 2 * _PL // nr
    for ci in range(_N_CORES):
        b, jq = divmod(ci, _JQ)
        o = np.asarray(res.results[ci]["out"])           # (NR, 128, 8*e)
        y = o.reshape(nr, 128, 8, e).transpose(2, 0, 1, 3)   # (slot, round, p, r)
        y = y.reshape(8, 2, _C_OUT, _TI, 32, 256)        # (slot, jc, c, i, jl, k)
        y = y.transpose(1, 0, 2, 3, 4, 5)                # (jc, slot, c, i, jl, k)
        for jc in range(2):
            for slot in range(8):
                dt, dh, dw = (slot >> 2) & 1, (slot >> 1) & 1, slot & 1
                tmp[
                    :, dt::2, jc * 64 + dh : jc * 64 + 64 : 2, dw::2
                ] = y[jc, slot]
        out[b, :, :, jq * 2 * _HJ : (jq + 1) * 2 * _HJ, :] = tmp[:, 1:]
    return out
